# revision 9
# baseline (speedup 1.0000x reference)
"""Trainium2 Bass kernel for nn_Part_Block (SE-style dynamic-weight CNN block).

Computation (per batch b):
    pooled = mean_hw x[b]                       (C,)
    hidden = silu(pooled @ fc1_w.T + fc1_b)     (C//16,) = (128,)
    dw     = (hidden @ fc2_w.T + fc2_b)         (P*C,) -> (P, C)
    base   = x[b] * conv_w + conv_b             (C, H, W)
    out    = softmax_p( einsum('chw,pc->phw', base, dw) )

Device strategy (8 cores, data-parallel over batch, 4 batches/core):
    - x[b] (4.5 MB) is DMA'd to SBUF once and stays resident; HBM read once.
    - conv folded into fc2:  logits[p,hw] = sum_c x[c,hw]*(conv_w[c]*dw[p,c]) + beta[p]
      with beta[p] = g[p,:] @ hidden + d[p],
           g[p,h]  = sum_c conv_b[c]*fc2_w[p*C+c,h],
           d[p]    = sum_c conv_b[c]*fc2_b[p*C+c]       (host precomputed)
    - fc2 emitted as 64 [128h x 128c] column-tiles so PE directly produces
      dwsT[c, p] per c-tile (the einsum lhsT) with no transpose.
    - einsum in fp32r (full PE rate at N>=256); fc1/fc2 in bf16.
    - softmax over p=4 via Exp(+beta bias) on ScalarE, ones-matmul column sum
      on PE, reciprocal + per-partition muls on DVE.
"""

from contextlib import ExitStack

import ml_dtypes
import numpy as np

import concourse.bass as bass
import concourse.mybir as mybir
import concourse.tile as tile
from concourse import bacc
from concourse.bass_utils import run_bass_kernel_spmd

N_CORES = 8
B, C, H, W = 32, 2048, 24, 24
HW = H * W                      # 576
P = 4                           # parts
CH = 128                        # hidden dim (C // 16)
B_LOC = B // N_CORES            # 4 batches per core
NT = C // 128                   # 16 c-tiles
NQ = P * NT                     # 64 fc2 column-tiles
CHUNK = NT // 4                 # c-tiles per DMA chunk (4)
NS = 288                        # einsum N split (576 = 2*288, both >=256)

F32 = mybir.dt.float32
F32R = mybir.dt.float32r
BF16 = mybir.dt.bfloat16

_BUILD_CACHE: dict = {}


def _build(repeat: int = 1):
    """Build + compile the SPMD single-core program (same on all 8 cores)."""
    nc = bacc.Bacc(
        "TRN2", target_bir_lowering=False, debug=False, num_devices=N_CORES
    )
    xs = nc.dram_tensor("xs", [B_LOC, C, HW], F32, kind="ExternalInput")
    fc1w_d = nc.dram_tensor("fc1w", [128, C], BF16, kind="ExternalInput")
    fc1b_d = nc.dram_tensor("fc1b", [128, 1], F32, kind="ExternalInput")
    fc2w_d = nc.dram_tensor("fc2w", [128, NQ * 128], BF16, kind="ExternalInput")
    fc2bs_d = nc.dram_tensor("fc2bs", [128, NQ], F32, kind="ExternalInput")
    gt_d = nc.dram_tensor("gt", [128, P], BF16, kind="ExternalInput")
    d_d = nc.dram_tensor("dvec", [P, 1], F32, kind="ExternalInput")
    ys = nc.dram_tensor("ys", [B_LOC, P, HW], F32, kind="ExternalOutput")

    with tile.TileContext(nc) as tc:
        with ExitStack() as ctx:
            const = ctx.enter_context(tc.tile_pool(name="const", bufs=1))
            xpool = ctx.enter_context(tc.tile_pool(name="x", bufs=3))
            work = ctx.enter_context(tc.tile_pool(name="work", bufs=2))
            psum = ctx.enter_context(tc.tile_pool(name="ps", bufs=1, space="PSUM"))

            fc1w = const.tile([128, C], BF16)
            nc.sync.dma_start(fc1w[:], fc1w_d.ap())
            fc1b = const.tile([128, 1], F32)
            nc.sync.dma_start(fc1b[:], fc1b_d.ap())
            fc2w = const.tile([128, NQ * 128], BF16)
            nc.sync.dma_start(fc2w[:], fc2w_d.ap())
            fc2bs = const.tile([128, NQ], F32)
            nc.sync.dma_start(fc2bs[:], fc2bs_d.ap())
            gt = const.tile([128, P], BF16)
            nc.sync.dma_start(gt[:], gt_d.ap())
            dvec = const.tile([P, 1], F32)
            nc.sync.dma_start(dvec[:], d_d.ap())
            ones4 = const.tile([P, 1], F32)
            nc.vector.memset(ones4[:], 1.0)
            ones1x4 = const.tile([1, P], F32)
            nc.vector.memset(ones1x4[:], 1.0)

            for _ in range(repeat):
                for b in range(B_LOC):
                    # ---- load x[b]: (t p) f -> p (t f), 4 chunks of 4 c-tiles
                    # fp32 -> bf16 cast happens inside the DMA (SWDGE).
                    x_t = xpool.tile([128, NT * HW], BF16)
                    xv = xs.ap()[b].rearrange("(t p) f -> p t f", p=128)
                    csz = CHUNK * HW  # 2304
                    for k in range(4):
                        nc.gpsimd.dma_start(
                            x_t[:, k * csz : (k + 1) * csz],
                            xv[:, k * CHUNK : (k + 1) * CHUNK, :],
                        )
                    # ---- pooling: sum over hw per c (scale 1/HW folded into fc1 act)
                    pooled = work.tile([128, NT], F32)
                    for k in range(4):
                        nc.vector.reduce_sum(
                            pooled[:, k * CHUNK : (k + 1) * CHUNK],
                            x_t[:, k * csz : (k + 1) * csz].rearrange(
                                "p (g f) -> p g f", f=HW
                            ),
                            mybir.AxisListType.X,
                        )
                    pooled_bf = work.tile([128, NT], BF16)
                    nc.vector.tensor_copy(pooled_bf[:], pooled[:])

                    # ---- fc1: hiddenT[h] = silu(sum_c pooled[c]*fc1_w[h,c]/HW + b)
                    # mm_ps col 0 = fc1 accumulation; col 1 = beta (shares bank)
                    mm_ps = psum.tile([128, 2], F32)
                    for t in range(NT):
                        nc.tensor.matmul(
                            mm_ps[:, 0:1],
                            lhsT=fc1w[:, t * 128 : (t + 1) * 128],
                            rhs=pooled_bf[:, t : t + 1],
                            start=(t == 0),
                            stop=(t == NT - 1),
                        )
                    hidden = work.tile([128, 1], BF16)
                    nc.scalar.activation(
                        hidden[:],
                        mm_ps[:, 0:1],
                        mybir.ActivationFunctionType.Silu,
                        bias=fc1b[:, 0:1],
                        scale=1.0 / HW,
                    )

                    # ---- fc2: dwsT[c, q=(t,p)] = fc2w_q.T @ hidden  (+ bias)
                    fc2_ps = psum.tile([128, NQ], F32)
                    for q in range(NQ):
                        nc.tensor.matmul(
                            fc2_ps[:, q : q + 1],
                            lhsT=fc2w[:, q * 128 : (q + 1) * 128],
                            rhs=hidden[:],
                            start=True,
                            stop=True,
                        )
                    dwst = work.tile([128, NQ], BF16)
                    nc.vector.tensor_add(dwst[:], fc2_ps[:], fc2bs[:])

                    # ---- beta[p] = gT.T @ hidden + d
                    nc.tensor.matmul(
                        mm_ps[0:P, 1:2], lhsT=gt[:], rhs=hidden[:], start=True, stop=True
                    )
                    beta = work.tile([P, 1], F32)
                    nc.scalar.activation(
                        beta[:],
                        mm_ps[0:P, 1:2],
                        mybir.ActivationFunctionType.Identity,
                        bias=dvec[:, 0:1],
                    )

                    # ---- einsum: logits[p, hw] = sum_t dwsT_t.T @ x_t  (bf16)
                    ps_a = psum.tile([P, NS], F32)
                    ps_b = psum.tile([P, NS], F32)
                    for t in range(NT):
                        lw = dwst[:, t * P : (t + 1) * P]
                        nc.tensor.matmul(
                            ps_a[:],
                            lhsT=lw,
                            rhs=x_t[:, t * HW : t * HW + NS],
                            start=(t == 0),
                            stop=(t == NT - 1),
                        )
                        nc.tensor.matmul(
                            ps_b[:],
                            lhsT=lw,
                            rhs=x_t[:, t * HW + NS : (t + 1) * HW],
                            start=(t == 0),
                            stop=(t == NT - 1),
                        )

                    # ---- softmax over p: e = exp(logits + beta)
                    e_sb = work.tile([P, HW], F32)
                    nc.scalar.activation(
                        e_sb[:, 0:NS],
                        ps_a[:],
                        mybir.ActivationFunctionType.Exp,
                        bias=beta[:, 0:1],
                    )
                    nc.scalar.activation(
                        e_sb[:, NS:HW],
                        ps_b[:],
                        mybir.ActivationFunctionType.Exp,
                        bias=beta[:, 0:1],
                    )
                    cs_a = psum.tile([1, NS], F32)
                    cs_b = psum.tile([1, NS], F32)
                    nc.tensor.matmul(
                        cs_a[:], lhsT=ones4[:], rhs=e_sb[:, 0:NS], start=True, stop=True
                    )
                    nc.tensor.matmul(
                        cs_b[:], lhsT=ones4[:], rhs=e_sb[:, NS:HW], start=True, stop=True
                    )
                    r_sb = work.tile([1, HW], F32)
                    nc.vector.reciprocal(r_sb[:, 0:NS], cs_a[:])
                    nc.vector.reciprocal(r_sb[:, NS:HW], cs_b[:])
                    # broadcast r over the 4 p-partitions via K=1 ones-matmul
                    r4_a = psum.tile([P, NS], F32)
                    r4_b = psum.tile([P, NS], F32)
                    nc.tensor.matmul(
                        r4_a[:], lhsT=ones1x4[:], rhs=r_sb[0:1, 0:NS],
                        start=True, stop=True,
                    )
                    nc.tensor.matmul(
                        r4_b[:], lhsT=ones1x4[:], rhs=r_sb[0:1, NS:HW],
                        start=True, stop=True,
                    )
                    out_sb = work.tile([P, HW], F32)
                    nc.vector.tensor_mul(out_sb[:, 0:NS], e_sb[:, 0:NS], r4_a[:])
                    nc.vector.tensor_mul(out_sb[:, NS:HW], e_sb[:, NS:HW], r4_b[:])
                    nc.sync.dma_start(ys.ap()[b], out_sb[:])
    nc.compile()
    return nc


def _host_prep(fc1_w, fc1_b, fc2_w, fc2_b, conv_w, conv_b):
    """Precompute device weight layouts on host (all small tensors)."""
    fc1_w = np.asarray(fc1_w, np.float32)
    fc1_b = np.asarray(fc1_b, np.float32)
    fc2_w = np.asarray(fc2_w, np.float32)
    fc2_b = np.asarray(fc2_b, np.float32)
    conv_w = np.asarray(conv_w, np.float32)
    conv_b = np.asarray(conv_b, np.float32)

    # fc1w[p, t*128+j] = fc1_w[j, t*128+p]
    fc1w = np.ascontiguousarray(
        fc1_w.reshape(128, NT, 128).transpose(2, 1, 0).reshape(128, C)
    ).astype(ml_dtypes.bfloat16)
    # fc2 scaled by conv_w over channel:  fc2_ws[i, h] = fc2_w[i, h]*conv_w[i % C]
    fc2_ws = fc2_w * np.tile(conv_w, P)[:, None]
    # fc2w[h, (t*4+p)*128 + c] = fc2_ws[p*C + t*128 + c, h]
    fc2wt = np.ascontiguousarray(
        fc2_ws.reshape(P, NT, 128, 128).transpose(3, 1, 0, 2).reshape(128, NQ * 128)
    ).astype(ml_dtypes.bfloat16)
    # fc2bs[c, t*4+p] = fc2_b[p*C + t*128 + c]*conv_w[t*128+c]
    fc2_bs = fc2_b * np.tile(conv_w, P)
    fc2bs = np.ascontiguousarray(
        fc2_bs.reshape(P, NT, 128).transpose(2, 1, 0).reshape(128, NQ)
    ).astype(np.float32)
    # g[p, h] = sum_c conv_b[c]*fc2_w[p*C+c, h];  d[p] = sum_c conv_b[c]*fc2_b[p*C+c]
    g = (fc2_w.reshape(P, C, 128).astype(np.float64) *
         conv_b.astype(np.float64)[None, :, None]).sum(axis=1)
    gt = np.ascontiguousarray(g.T).astype(ml_dtypes.bfloat16)
    d = (fc2_b.reshape(P, C).astype(np.float64) @ conv_b.astype(np.float64))
    dvec = d.astype(np.float32).reshape(P, 1)
    return {
        "fc1w": fc1w,
        "fc1b": fc1_b.reshape(128, 1),
        "fc2w": fc2wt,
        "fc2bs": fc2bs,
        "gt": gt,
        "dvec": dvec,
    }


def _run(in_maps, repeat: int = 1):
    if repeat not in _BUILD_CACHE:
        _BUILD_CACHE[repeat] = _build(repeat)
    nc = _BUILD_CACHE[repeat]
    return run_bass_kernel_spmd(nc, in_maps, list(range(N_CORES)))


def make_in_maps(x, weights):
    x3 = np.ascontiguousarray(np.asarray(x, np.float32).reshape(B, C, HW))
    return [
        {"xs": x3[i * B_LOC : (i + 1) * B_LOC], **weights} for i in range(N_CORES)
    ]


def kernel(x, fc1_w, fc1_b, fc2_w, fc2_b, conv_w, conv_b):
    weights = _host_prep(fc1_w, fc1_b, fc2_w, fc2_b, conv_w, conv_b)
    in_maps = make_in_maps(x, weights)
    res = _run(in_maps, repeat=1)
    out = np.concatenate(
        [res.results[i]["ys"] for i in range(N_CORES)], axis=0
    )
    return np.ascontiguousarray(out.reshape(B, P, H, W).astype(np.float32))


# revision 13
# speedup vs baseline: 2.4470x; 2.4470x over previous
"""Trainium2 Bass kernel for nn_Part_Block (SE-style dynamic-weight CNN block).

Computation (per batch b):
    pooled = mean_hw x[b]                       (C,)
    hidden = silu(pooled @ fc1_w.T + fc1_b)     (C//16,) = (128,)
    dw     = (hidden @ fc2_w.T + fc2_b)         (P*C,) -> (P, C)
    base   = x[b] * conv_w + conv_b             (C, H, W)
    out    = softmax_p( einsum('chw,pc->phw', base, dw) )

Device strategy (8 cores, data-parallel over batch, 4 batches/core):
    - x[b] (4.5 MB) is DMA'd to SBUF once and stays resident; HBM read once.
    - conv folded into fc2:  logits[p,hw] = sum_c x[c,hw]*(conv_w[c]*dw[p,c]) + beta[p]
      with beta[p] = g[p,:] @ hidden + d[p],
           g[p,h]  = sum_c conv_b[c]*fc2_w[p*C+c,h],
           d[p]    = sum_c conv_b[c]*fc2_b[p*C+c]       (host precomputed)
    - fc2 emitted as 64 [128h x 128c] column-tiles so PE directly produces
      dwsT[c, p] per c-tile (the einsum lhsT) with no transpose.
    - einsum in fp32r (full PE rate at N>=256); fc1/fc2 in bf16.
    - softmax over p=4 via Exp(+beta bias) on ScalarE, ones-matmul column sum
      on PE, reciprocal + per-partition muls on DVE.
"""

from contextlib import ExitStack

import ml_dtypes
import numpy as np

import concourse.bass as bass
import concourse.mybir as mybir
import concourse.tile as tile
from concourse import bacc
from concourse.bass_utils import run_bass_kernel_spmd

N_CORES = 8
B, C, H, W = 32, 2048, 24, 24
HW = H * W                      # 576
P = 4                           # parts
CH = 128                        # hidden dim (C // 16)
B_LOC = B // N_CORES            # 4 batches per core
NT = C // 128                   # 16 c-tiles
NQ = P * NT                     # 64 fc2 column-tiles
CHUNK = NT // 4                 # c-tiles per DMA chunk (4)
NS = 288                        # einsum N split (576 = 2*288, both >=256)

F32 = mybir.dt.float32
F32R = mybir.dt.float32r
BF16 = mybir.dt.bfloat16

_BUILD_CACHE: dict = {}


def _build(repeat: int = 1):
    """Build + compile the SPMD single-core program (same on all 8 cores)."""
    nc = bacc.Bacc(
        "TRN2", target_bir_lowering=False, debug=False, num_devices=N_CORES
    )
    xs = nc.dram_tensor("xs", [B_LOC, C, HW], F32, kind="ExternalInput")
    fc1w_d = nc.dram_tensor("fc1w", [128, C], BF16, kind="ExternalInput")
    fc1b_d = nc.dram_tensor("fc1b", [128, 1], F32, kind="ExternalInput")
    fc2w_d = nc.dram_tensor("fc2w", [128, NQ * 128], BF16, kind="ExternalInput")
    fc2bs_d = nc.dram_tensor("fc2bs", [128, NQ], F32, kind="ExternalInput")
    gt_d = nc.dram_tensor("gt", [128, P], BF16, kind="ExternalInput")
    d_d = nc.dram_tensor("dvec", [P, 1], F32, kind="ExternalInput")
    ys = nc.dram_tensor("ys", [B_LOC, P, HW], F32, kind="ExternalOutput")

    with tile.TileContext(nc) as tc:
        with ExitStack() as ctx:
            const = ctx.enter_context(tc.tile_pool(name="const", bufs=1))
            xpool = ctx.enter_context(tc.tile_pool(name="x", bufs=2))
            xbfpool = ctx.enter_context(tc.tile_pool(name="xbf", bufs=2))
            work = ctx.enter_context(tc.tile_pool(name="work", bufs=2))
            psum = ctx.enter_context(tc.tile_pool(name="ps", bufs=1, space="PSUM"))

            fc1w = const.tile([128, C], BF16)
            nc.sync.dma_start(fc1w[:], fc1w_d.ap())
            fc1b = const.tile([128, 1], F32)
            nc.sync.dma_start(fc1b[:], fc1b_d.ap())
            fc2w = const.tile([128, NQ * 128], BF16)
            nc.sync.dma_start(fc2w[:], fc2w_d.ap())
            fc2bs = const.tile([128, NQ], F32)
            nc.sync.dma_start(fc2bs[:], fc2bs_d.ap())
            gt = const.tile([128, P], BF16)
            nc.sync.dma_start(gt[:], gt_d.ap())
            dvec = const.tile([P, 1], F32)
            nc.sync.dma_start(dvec[:], d_d.ap())
            ones4 = const.tile([P, 1], F32)
            nc.vector.memset(ones4[:], 1.0)
            ones1x4 = const.tile([1, P], F32)
            nc.vector.memset(ones1x4[:], 1.0)

            for _ in range(repeat):
                for b in range(B_LOC):
                    # ---- load x[b] fp32 via HWDGE: (t p) f -> p (t f), 4 chunks
                    x_t = xpool.tile([128, NT * HW], F32)
                    xv = xs.ap()[b].rearrange("(t p) f -> p t f", p=128)
                    csz = CHUNK * HW  # 2304
                    for k in range(4):
                        nc.sync.dma_start(
                            x_t[:, k * csz : (k + 1) * csz],
                            xv[:, k * CHUNK : (k + 1) * CHUNK, :],
                        )
                    # ---- fused cast fp32->bf16 + pooling (accum_out = row sum),
                    # alternating ScalarE / VectorE per c-tile
                    x_bf = xbfpool.tile([128, NT * HW], BF16)
                    pooled = work.tile([128, NT], F32)
                    for t in range(NT):
                        src = x_t[:, t * HW : (t + 1) * HW]
                        dst = x_bf[:, t * HW : (t + 1) * HW]
                        acc = pooled[:, t : t + 1]
                        if t % 2 == 0:
                            nc.scalar.activation(
                                dst,
                                src,
                                mybir.ActivationFunctionType.Copy,
                                accum_out=acc,
                            )
                        else:
                            nc.vector.tensor_scalar(
                                dst,
                                src,
                                1.0,
                                0.0,
                                mybir.AluOpType.mult,
                                mybir.AluOpType.add,
                                accum_out=acc,
                            )
                    pooled_bf = work.tile([128, NT], BF16)
                    nc.vector.tensor_copy(pooled_bf[:], pooled[:])

                    # ---- fc1: hiddenT[h] = silu(sum_c pooled[c]*fc1_w[h,c]/HW + b)
                    # mm_ps col 0 = fc1 accumulation; col 1 = beta (shares bank)
                    mm_ps = psum.tile([128, 2], F32)
                    for t in range(NT):
                        nc.tensor.matmul(
                            mm_ps[:, 0:1],
                            lhsT=fc1w[:, t * 128 : (t + 1) * 128],
                            rhs=pooled_bf[:, t : t + 1],
                            start=(t == 0),
                            stop=(t == NT - 1),
                        )
                    hidden = work.tile([128, 1], BF16)
                    nc.scalar.activation(
                        hidden[:],
                        mm_ps[:, 0:1],
                        mybir.ActivationFunctionType.Silu,
                        bias=fc1b[:, 0:1],
                        scale=1.0 / HW,
                    )

                    # ---- fc2: dwsT[c, q=(t,p)] = fc2w_q.T @ hidden  (+ bias)
                    fc2_ps = psum.tile([128, NQ], F32)
                    for q in range(NQ):
                        nc.tensor.matmul(
                            fc2_ps[:, q : q + 1],
                            lhsT=fc2w[:, q * 128 : (q + 1) * 128],
                            rhs=hidden[:],
                            start=True,
                            stop=True,
                        )
                    dwst = work.tile([128, NQ], BF16)
                    nc.vector.tensor_add(dwst[:], fc2_ps[:], fc2bs[:])

                    # ---- beta[p] = gT.T @ hidden + d
                    nc.tensor.matmul(
                        mm_ps[0:P, 1:2], lhsT=gt[:], rhs=hidden[:], start=True, stop=True
                    )
                    beta = work.tile([P, 1], F32)
                    nc.scalar.activation(
                        beta[:],
                        mm_ps[0:P, 1:2],
                        mybir.ActivationFunctionType.Identity,
                        bias=dvec[:, 0:1],
                    )

                    # ---- einsum: logits[p, hw] = sum_t dwsT_t.T @ x_t  (bf16)
                    ps_a = psum.tile([P, NS], F32)
                    ps_b = psum.tile([P, NS], F32)
                    for t in range(NT):
                        lw = dwst[:, t * P : (t + 1) * P]
                        nc.tensor.matmul(
                            ps_a[:],
                            lhsT=lw,
                            rhs=x_bf[:, t * HW : t * HW + NS],
                            start=(t == 0),
                            stop=(t == NT - 1),
                        )
                        nc.tensor.matmul(
                            ps_b[:],
                            lhsT=lw,
                            rhs=x_bf[:, t * HW + NS : (t + 1) * HW],
                            start=(t == 0),
                            stop=(t == NT - 1),
                        )

                    # ---- softmax over p: e = exp(logits + beta)
                    e_sb = work.tile([P, HW], F32)
                    nc.scalar.activation(
                        e_sb[:, 0:NS],
                        ps_a[:],
                        mybir.ActivationFunctionType.Exp,
                        bias=beta[:, 0:1],
                    )
                    nc.scalar.activation(
                        e_sb[:, NS:HW],
                        ps_b[:],
                        mybir.ActivationFunctionType.Exp,
                        bias=beta[:, 0:1],
                    )
                    cs_a = psum.tile([1, NS], F32)
                    cs_b = psum.tile([1, NS], F32)
                    nc.tensor.matmul(
                        cs_a[:], lhsT=ones4[:], rhs=e_sb[:, 0:NS], start=True, stop=True
                    )
                    nc.tensor.matmul(
                        cs_b[:], lhsT=ones4[:], rhs=e_sb[:, NS:HW], start=True, stop=True
                    )
                    r_sb = work.tile([1, HW], F32)
                    nc.vector.reciprocal(r_sb[:, 0:NS], cs_a[:])
                    nc.vector.reciprocal(r_sb[:, NS:HW], cs_b[:])
                    # broadcast r over the 4 p-partitions via K=1 ones-matmul
                    r4_a = psum.tile([P, NS], F32)
                    r4_b = psum.tile([P, NS], F32)
                    nc.tensor.matmul(
                        r4_a[:], lhsT=ones1x4[:], rhs=r_sb[0:1, 0:NS],
                        start=True, stop=True,
                    )
                    nc.tensor.matmul(
                        r4_b[:], lhsT=ones1x4[:], rhs=r_sb[0:1, NS:HW],
                        start=True, stop=True,
                    )
                    out_sb = work.tile([P, HW], F32)
                    nc.vector.tensor_mul(out_sb[:, 0:NS], e_sb[:, 0:NS], r4_a[:])
                    nc.vector.tensor_mul(out_sb[:, NS:HW], e_sb[:, NS:HW], r4_b[:])
                    nc.sync.dma_start(ys.ap()[b], out_sb[:])
    nc.compile()
    return nc


def _host_prep(fc1_w, fc1_b, fc2_w, fc2_b, conv_w, conv_b):
    """Precompute device weight layouts on host (all small tensors)."""
    fc1_w = np.asarray(fc1_w, np.float32)
    fc1_b = np.asarray(fc1_b, np.float32)
    fc2_w = np.asarray(fc2_w, np.float32)
    fc2_b = np.asarray(fc2_b, np.float32)
    conv_w = np.asarray(conv_w, np.float32)
    conv_b = np.asarray(conv_b, np.float32)

    # fc1w[p, t*128+j] = fc1_w[j, t*128+p]
    fc1w = np.ascontiguousarray(
        fc1_w.reshape(128, NT, 128).transpose(2, 1, 0).reshape(128, C)
    ).astype(ml_dtypes.bfloat16)
    # fc2 scaled by conv_w over channel:  fc2_ws[i, h] = fc2_w[i, h]*conv_w[i % C]
    fc2_ws = fc2_w * np.tile(conv_w, P)[:, None]
    # fc2w[h, (t*4+p)*128 + c] = fc2_ws[p*C + t*128 + c, h]
    fc2wt = np.ascontiguousarray(
        fc2_ws.reshape(P, NT, 128, 128).transpose(3, 1, 0, 2).reshape(128, NQ * 128)
    ).astype(ml_dtypes.bfloat16)
    # fc2bs[c, t*4+p] = fc2_b[p*C + t*128 + c]*conv_w[t*128+c]
    fc2_bs = fc2_b * np.tile(conv_w, P)
    fc2bs = np.ascontiguousarray(
        fc2_bs.reshape(P, NT, 128).transpose(2, 1, 0).reshape(128, NQ)
    ).astype(np.float32)
    # g[p, h] = sum_c conv_b[c]*fc2_w[p*C+c, h];  d[p] = sum_c conv_b[c]*fc2_b[p*C+c]
    g = (fc2_w.reshape(P, C, 128).astype(np.float64) *
         conv_b.astype(np.float64)[None, :, None]).sum(axis=1)
    gt = np.ascontiguousarray(g.T).astype(ml_dtypes.bfloat16)
    d = (fc2_b.reshape(P, C).astype(np.float64) @ conv_b.astype(np.float64))
    dvec = d.astype(np.float32).reshape(P, 1)
    return {
        "fc1w": fc1w,
        "fc1b": fc1_b.reshape(128, 1),
        "fc2w": fc2wt,
        "fc2bs": fc2bs,
        "gt": gt,
        "dvec": dvec,
    }


def _run(in_maps, repeat: int = 1):
    if repeat not in _BUILD_CACHE:
        _BUILD_CACHE[repeat] = _build(repeat)
    nc = _BUILD_CACHE[repeat]
    return run_bass_kernel_spmd(nc, in_maps, list(range(N_CORES)))


def make_in_maps(x, weights):
    x3 = np.ascontiguousarray(np.asarray(x, np.float32).reshape(B, C, HW))
    return [
        {"xs": x3[i * B_LOC : (i + 1) * B_LOC], **weights} for i in range(N_CORES)
    ]


def kernel(x, fc1_w, fc1_b, fc2_w, fc2_b, conv_w, conv_b):
    weights = _host_prep(fc1_w, fc1_b, fc2_w, fc2_b, conv_w, conv_b)
    in_maps = make_in_maps(x, weights)
    res = _run(in_maps, repeat=1)
    out = np.concatenate(
        [res.results[i]["ys"] for i in range(N_CORES)], axis=0
    )
    return np.ascontiguousarray(out.reshape(B, P, H, W).astype(np.float32))
